# revision 1
# baseline (speedup 1.0000x reference)
"""Trainium2 Bass kernel for nn_CustomLoss (argmax-distance weighted loss).

reference:
    arg = argmax(target, axis=1)              # [B]
    delta = distance[arg]                     # [B]
    err = |distance[None,:] - delta[:,None]| + 1
    loss = sum((output - target) * err) / B

Algorithm (no gathers, data-parallel over 8 NeuronCores):
  With dist = [-0.5, -0.34, 0, 0.34, 0.5] and e_a = [t_a >= max_c t_c]:
    2*delta = (e4 - e0) + 0.68*(e3 - e1)          (dist[2]=0 -> e2 unused;
                                                   argmax==2 gives w2=0=2*dist[2])
    err[b,c] + 1 -> (|2*delta - 2*dist_c| + 2)/2
  loss*2*B = sum over b,c of (o - t) * (|w2 - 2*dist_c| + 2)

Per-core layout: rows on 128 partitions, 5 classes interleaved along free dim,
8 tiles of [128, 2560].  Engines: DMA loads t (f32, HWDGE) + o (bf16 cast,
SWDGE); GPSIMD does the 4-op max tree; VectorE does compares and the fused
(w+2)*d product with per-partition accum; ScalarE does the bf16 cast and the
5 Abs activations.  Output: [128, ntiles] partial sums per core, reduced on
host and divided by 2*B.
"""

from contextlib import ExitStack

import numpy as np

P = 128
C = 5
DIST = (-0.5, -0.34, 0.0, 0.34, 0.5)
B = 4194304
NCORES = 8
ROWS_PER_CORE = B // NCORES  # 524288
G = 512                      # rows per partition per tile
NTILES = ROWS_PER_CORE // (P * G)  # 8

_CACHE = {}


def _build_nc():
    import concourse.bacc as bacc
    import concourse.mybir as mybir
    import concourse.tile as tile

    F32 = mybir.dt.float32
    BF16 = mybir.dt.bfloat16
    FREE = C * G

    nc = bacc.Bacc(target_bir_lowering=False)

    # Register activation-bias constants (-2*dist[c]) in the const-AP database,
    # mirroring what Bass.__init__ does for 0.0/1.0.
    for c in range(C):
        val = -2.0 * DIST[c]
        if (F32, val) not in nc.const_aps.aps:
            tensor = nc.alloc_sbuf_tensor(f"const-f32-bias{c}", [P, 1], F32)
            nc.gpsimd.memset(tensor.ap(), val)
            nc.const_aps.aps[(F32, val)] = tensor.ap()
    nc.all_engine_barrier()

    t_in = nc.declare_dram_parameter("t", [ROWS_PER_CORE, C], F32, isOutput=False)
    o_in = nc.declare_dram_parameter("o", [ROWS_PER_CORE, C], F32, isOutput=False)
    out = nc.declare_dram_parameter("out", [1, 2 * G], F32, isOutput=True)

    # row = n*(P*G) + p*G + g ; per-partition data is contiguous in DRAM
    t_tiled = t_in.rearrange("(n p g) c -> n p (g c)", p=P, g=G)
    o_tiled = o_in.rearrange("(n p g) c -> n p (g c)", p=P, g=G)

    ones_bf16 = nc.const_aps.aps[(BF16, 1.0)]  # [128, 1] of 1.0, preregistered

    with ExitStack() as ctx:
        tc = ctx.enter_context(tile.TileContext(nc))
        pool = ctx.enter_context(tc.tile_pool(name="work", bufs=3))
        psp = ctx.enter_context(tc.tile_pool(name="ps", bufs=1, space="PSUM"))
        outp = ctx.enter_context(tc.tile_pool(name="outp", bufs=1))
        ps_p = psp.tile([1, G], F32)   # sum of wI*d
        ps_d = psp.tile([1, G], F32)   # sum of d

        # Software-pipelined emission: tile k's front work (loads, cast, max,
        # compares) is emitted before tile k-1's back work (Abs weights,
        # product, matmuls) so each engine's in-order stream has cross-tile
        # lookahead and DVE never stalls on ScalarE's Abs chain.
        state = {}

        def emit_front(k):
            t = pool.tile([P, FREE], F32, tag="t", name="t", bufs=3)
            nc.sync.dma_start(t[:, :], t_tiled[k])
            o = pool.tile([P, FREE], BF16, tag="o", name="o", bufs=3)
            nc.gpsimd.dma_start(o[:, :], o_tiled[k])  # f32 -> bf16 cast in DMA

            tb = pool.tile([P, FREE], BF16, tag="tb", name="tb", bufs=3)
            nc.scalar.copy(tb[:, :], t[:, :])  # ACT cast f32->bf16

            tv = t[:, :].rearrange("p (g c) -> p g c", c=C)

            # m[p,g] = max over the 5 classes (segmented reduce, unit-stride)
            m = pool.tile([P, G], F32, tag="m", name="m", bufs=4)
            nc.vector.tensor_reduce(
                m[:, :], tv, axis=mybir.AxisListType.X, op=mybir.AluOpType.max
            )

            # E[p,g,c] = [t >= m]  (one pass, m broadcast along class dim)
            E = pool.tile([P, FREE], BF16, tag="E", name="E", bufs=3)
            nc.vector.tensor_tensor(
                E[:, :].rearrange("p (g c) -> p g c", c=C),
                tv,
                m[:, :].to_broadcast([P, G, C]),
                op=mybir.AluOpType.is_ge,
            )

            Ev = E[:, :].rearrange("p (g c) -> p g c", c=C)
            u = pool.tile([P, G], BF16, tag="u", name="u", bufs=4)
            nc.vector.tensor_sub(u[:, :], Ev[:, :, 4], Ev[:, :, 0])
            v = pool.tile([P, G], BF16, tag="v", name="v", bufs=4)
            nc.vector.tensor_sub(v[:, :], Ev[:, :, 3], Ev[:, :, 1])
            w2 = pool.tile([P, G], BF16, tag="w2", name="w2", bufs=4)
            # w2 = (v * 0.68) + u  == 2*delta
            nc.vector.scalar_tensor_tensor(
                w2[:, :], v[:, :], 0.68, u[:, :],
                mybir.AluOpType.mult, mybir.AluOpType.add,
            )

            d = pool.tile([P, FREE], BF16, tag="d", name="d", bufs=4)
            nc.vector.tensor_sub(d[:, :], o[:, :], tb[:, :])
            state[k] = (w2, d)

        def emit_back(k):
            w2, d = state.pop(k)
            # wI[:, g, c] = |w2 - 2*dist[c]|  (ScalarE)
            wI = pool.tile([P, FREE], BF16, tag="wI", name="wI", bufs=3)
            wIv = wI[:, :].rearrange("p (g c) -> p g c", c=C)
            for c in range(C):
                nc.scalar.activation(
                    wIv[:, :, c], w2[:, :], mybir.ActivationFunctionType.Abs,
                    bias=-2.0 * DIST[c], scale=1.0,
                )

            # p = wI * d  (bf16 2x mode), then TensorE ones-matmul reduces
            # p and d into PSUM accumulators across all tiles
            p = pool.tile([P, FREE], BF16, tag="p", name="p", bufs=3)
            nc.vector.tensor_mul(p[:, :], wI[:, :], d[:, :])
            for j in range(C):
                first = k == 0 and j == 0
                last = k == NTILES - 1 and j == C - 1
                nc.tensor.matmul(
                    ps_p[:, :], ones_bf16, p[:, j * G : (j + 1) * G],
                    start=first, stop=last,
                )
                nc.tensor.matmul(
                    ps_d[:, :], ones_bf16, d[:, j * G : (j + 1) * G],
                    start=first, stop=last,
                )

        for k in range(NTILES):
            emit_front(k)
            if k >= 1:
                emit_back(k - 1)
        emit_back(NTILES - 1)

        # readout: [1, 2G] f32 -> DRAM; host computes (sum0 + 2*sum1) / 2B
        res = outp.tile([1, 2 * G], F32)
        nc.scalar.copy(res[:, 0:G], ps_p[:, :])
        nc.scalar.copy(res[:, G : 2 * G], ps_d[:, :])
        nc.sync.dma_start(out[:, :], res[:, :])
    nc.finalize()
    return nc


def _get_nc():
    if "nc" not in _CACHE:
        _CACHE["nc"] = _build_nc()
    return _CACHE["nc"]


def kernel(output, target, distance, _want_results=False):
    from concourse.bass_utils import run_bass_kernel_spmd

    output = np.asarray(output, dtype=np.float32)
    target = np.asarray(target, dtype=np.float32)
    distance = np.asarray(distance, dtype=np.float32)
    assert output.shape == (B, C) and target.shape == (B, C)
    assert np.allclose(distance, np.asarray(DIST, np.float32)), distance

    nc = _get_nc()
    o_sh = output.reshape(NCORES, ROWS_PER_CORE, C)
    t_sh = target.reshape(NCORES, ROWS_PER_CORE, C)
    in_maps = [
        {"t": np.ascontiguousarray(t_sh[i]), "o": np.ascontiguousarray(o_sh[i])}
        for i in range(NCORES)
    ]
    res = run_bass_kernel_spmd(nc, in_maps, core_ids=list(range(NCORES)))
    total = 0.0
    for r in res.results:
        arr = r["out"].astype(np.float64).reshape(2, G)
        total += float(arr[0].sum() + 2.0 * arr[1].sum())
    loss = np.float32(total / 2.0 / B)
    if _want_results:
        return loss, res
    return loss



# revision 8
# speedup vs baseline: 1.2491x; 1.2491x over previous
"""Trainium2 Bass kernel for nn_CustomLoss (argmax-distance weighted loss).

reference:
    arg = argmax(target, axis=1)              # [B]
    delta = distance[arg]                     # [B]
    err = |distance[None,:] - delta[:,None]| + 1
    loss = sum((output - target) * err) / B

v3 design (data-parallel over 8 NeuronCores):
  Host: per-core slice, permute classes to PERM=(4,3,0,1,2), lay out DRAM
  exactly in tile order [NT, P, (c g)] and cast f32->bf16 (halves HBM
  traffic, doubles DVE throughput; bf16 argmax ties cost ~5e-4 rel err).

  With one-hot E_c = [t_c >= m], m = max_c t_c, block order (c4,c3,c0,c1,c2):
    u = E_c4 - E_c0 = blk0 - blk2,  v = E_c3 - E_c1 = blk1 - blk3
    (one fused subtract on adjacent block pairs -> uv)
    w2 = 2*delta = 0.68*v + u
    wI_b = |w2 - 2*dist_b|  (ScalarE Abs, contiguous per block)
    loss*2B = sum((wI+2)*d),  d = o - t

  Engines: DVE: max tree (h,hm,m), E=is_ge, uv, w2, d = o - t,
  p = (wI+2)*d via scalar_tensor_tensor with accum_out (sum per partition;
  full product discarded through a stride-0 dummy).  ScalarE: o DMA +
  5 Abs.  Sync: t DMA.
  Readout: acc [128, NT] f32 -> DRAM; host sums in f64, / (2B).
"""

from contextlib import ExitStack

import numpy as np

P = 128
C = 5
DIST = (-0.5, -0.34, 0.0, 0.34, 0.5)
B = 4194304
NCORES = 8
ROWS_PER_CORE = B // NCORES  # 524288
NTILES = 2
G = ROWS_PER_CORE // (P * NTILES)   # rows/partition/class-block/tile
FREE = C * G

# class order in device layout; block b holds class PERM[b]
PERM = (4, 3, 0, 1, 2)
BIAS = tuple(-2.0 * DIST[c] for c in PERM)  # (-1.0, -0.68, 1.0, 0.68, -0.0)

O_DMA_ENGINE = "scalar"  # "sync" | "scalar"
DUMMY_OUT = True

_CACHE = {}


def _build_nc():
    import concourse.bacc as bacc
    import concourse.mybir as mybir
    import concourse.tile as tile

    F32 = mybir.dt.float32
    BF16 = mybir.dt.bfloat16

    nc = bacc.Bacc(target_bir_lowering=False)

    # Register activation-bias constants in the const-AP database (f32 keys,
    # as const_aps.scalar_like expects), mirroring Bass.__init__'s 0.0/1.0.
    for val in BIAS:
        if (F32, val) not in nc.const_aps.aps:
            tensor = nc.alloc_sbuf_tensor(f"const-f32-b{val}", [P, 1], F32)
            nc.gpsimd.memset(tensor.ap(), val)
            nc.const_aps.aps[(F32, val)] = tensor.ap()
    nc.all_engine_barrier()

    # host pre-arranges DRAM exactly in tile layout: [tile, partition, (c g)]
    t_in = nc.declare_dram_parameter("t", [NTILES, P, FREE], BF16, isOutput=False)
    o_in = nc.declare_dram_parameter("o", [NTILES, P, FREE], BF16, isOutput=False)
    out = nc.declare_dram_parameter("out", [P, NTILES], F32, isOutput=True)

    bufs_big = 2 if NTILES == 2 else 3

    with ExitStack() as ctx:
        tc = ctx.enter_context(tile.TileContext(nc))
        pool = ctx.enter_context(tc.tile_pool(name="work", bufs=1))
        outp = ctx.enter_context(tc.tile_pool(name="outp", bufs=1))
        acc = outp.tile([P, NTILES], F32)
        dummy = outp.tile([P, 1], BF16)

        st = {}

        def phase_load(k):
            t = pool.tile([P, FREE], BF16, tag="t", name="t", bufs=bufs_big)
            nc.sync.dma_start(t[:, :], t_in[k])
            o = pool.tile([P, FREE], BF16, tag="o", name="o", bufs=bufs_big)
            eng = nc.scalar if O_DMA_ENGINE == "scalar" else nc.sync
            eng.dma_start(o[:, :], o_in[k])
            st[k] = {"t": t, "o": o}

        def phase_front(k):
            s = st[k]
            t, o = s["t"], s["o"]
            TT = nc.vector.tensor_tensor
            MAX = mybir.AluOpType.max

            # m = max over the 5 class blocks
            h = pool.tile([P, 2 * G], BF16, tag="h", name="h", bufs=1)
            TT(h[:, :], t[:, 0 : 2 * G], t[:, 2 * G : 4 * G], op=MAX)
            hm = pool.tile([P, G], BF16, tag="hm", name="hm", bufs=1)
            TT(hm[:, :], h[:, 0:G], h[:, G : 2 * G], op=MAX)
            m = pool.tile([P, G], BF16, tag="m", name="m", bufs=1)
            TT(m[:, :], hm[:, :], t[:, 4 * G : 5 * G], op=MAX)

            # E[b] = [t_b >= m] for blocks 0..3 (c4,c3,c0,c1)
            E = pool.tile([P, 4 * G], BF16, tag="E", name="E", bufs=1)
            mv = m[:, :].rearrange("p (x g) -> p x g", x=1)
            TT(
                E[:, :].rearrange("p (c g) -> p c g", g=G),
                t[:, 0 : 4 * G].rearrange("p (c g) -> p c g", g=G),
                mv.to_broadcast([P, 4, G]),
                op=mybir.AluOpType.is_ge,
            )

            # uv = (E_c4 - E_c0, E_c3 - E_c1) in one op on block pairs
            uv = pool.tile([P, 2 * G], BF16, tag="uv", name="uv", bufs=1)
            TT(
                uv[:, :], E[:, 0 : 2 * G], E[:, 2 * G : 4 * G],
                op=mybir.AluOpType.subtract,
            )

            w2 = pool.tile([P, G], BF16, tag="w2", name="w2", bufs=2)
            nc.vector.scalar_tensor_tensor(
                w2[:, :], uv[:, G : 2 * G], 0.68, uv[:, 0:G],
                mybir.AluOpType.mult, mybir.AluOpType.add,
            )

            # wI[b] = |w2 + bias_b|, contiguous per block (ScalarE)
            wI = pool.tile([P, FREE], BF16, tag="wI", name="wI", bufs=1)
            for c in range(C):
                nc.scalar.activation(
                    wI[:, c * G : (c + 1) * G], w2[:, :],
                    mybir.ActivationFunctionType.Abs,
                    bias=BIAS[c], scale=1.0,
                )
            s["wI"] = wI

            # d = o - t
            d = pool.tile([P, FREE], BF16, tag="d", name="d", bufs=2)
            nc.vector.tensor_tensor(
                d[:, :], o[:, :], t[:, :], op=mybir.AluOpType.subtract
            )
            s["d"] = d

        def phase_back(k):
            s = st.pop(k)
            wI, d = s["wI"], s["d"]
            # acc[:, k] = sum((wI + 2) * d); product discarded via dummy
            if DUMMY_OUT:
                p_out = dummy[:, :].broadcast_to([P, FREE])
            else:
                p_out = pool.tile([P, FREE], BF16, tag="p", name="p", bufs=2)[:, :]
            nc.vector.scalar_tensor_tensor(
                p_out,
                wI[:, :], 2.0, d[:, :],
                mybir.AluOpType.add, mybir.AluOpType.mult,
                accum_out=acc[:, k : k + 1],
            )

        for k in range(NTILES):
            phase_load(k)
        for k in range(NTILES):
            phase_front(k)
        for k in range(NTILES):
            phase_back(k)

        nc.sync.dma_start(out[:, :], acc[:, :])
    nc.finalize()
    return nc


def _get_nc():
    if "nc" not in _CACHE:
        _CACHE["nc"] = _build_nc()
    return _CACHE["nc"]


def _prep_inputs(output, target):
    """Per-core tile-layout bf16 arrays: [NT, P, (c g)], classes permuted."""
    from ml_dtypes import bfloat16

    def lay(x_core):
        # rows r = (n*P + p)*G + g, classes reordered by PERM
        x = x_core[:, list(PERM)].reshape(NTILES, P, G, C)
        x = np.ascontiguousarray(x.transpose(0, 1, 3, 2)).reshape(NTILES, P, FREE)
        return x.astype(bfloat16)

    o_sh = output.reshape(NCORES, ROWS_PER_CORE, C)
    t_sh = target.reshape(NCORES, ROWS_PER_CORE, C)
    return [{"t": lay(t_sh[i]), "o": lay(o_sh[i])} for i in range(NCORES)]


def reduce_loss(res):
    total = 0.0
    for r in res.results:
        total += float(r["out"].astype(np.float64).sum())
    return total / 2.0 / B


def kernel(output, target, distance, _want_results=False):
    from concourse.bass_utils import run_bass_kernel_spmd

    output = np.asarray(output, dtype=np.float32)
    target = np.asarray(target, dtype=np.float32)
    distance = np.asarray(distance, dtype=np.float32)
    assert output.shape == (B, C) and target.shape == (B, C)
    assert np.allclose(distance, np.asarray(DIST, np.float32)), distance

    nc = _get_nc()
    in_maps = _prep_inputs(output, target)
    res = run_bass_kernel_spmd(nc, in_maps, core_ids=list(range(NCORES)))
    loss = np.float32(reduce_loss(res))
    if _want_results:
        return loss, res
    return loss


# revision 10
# speedup vs baseline: 1.4402x; 1.1530x over previous
"""Trainium2 Bass kernel for nn_CustomLoss (argmax-distance weighted loss).

reference:
    arg = argmax(target, axis=1)              # [B]
    delta = distance[arg]                     # [B]
    err = |distance[None,:] - delta[:,None]| + 1
    loss = sum((output - target) * err) / B

v5 design (data-parallel over 8 NeuronCores):
  Host: per-core slice, permute classes to PERM=(4,3,0,1,2), lay out DRAM
  in tile order [P, (c g)] per tile, cast f32->bf16 (halves HBM traffic,
  doubles DVE throughput; bf16 argmax ties cost ~5e-4 rel err), and
  pre-negate o so the device can form d via DMA accum-add.

  With one-hot E_c = [t_c >= m], m = max_c t_c, block order (c4,c3,c0,c1,c2):
    u = E_c4 - E_c0 = blk0 - blk2,  v = E_c3 - E_c1 = blk1 - blk3
    w2 = 2*delta = 0.68*v + u
    wI_b = |w2 - 2*dist_b|  (= 2*|delta - dist_b|)
    dneg = t - o  (= -(o - t))
    loss*B = -(0.5*sum(wI*dneg) + sum(dneg))

  Engines:
    sync HWDGE:   t loads
    scalar HWDGE: (-o) loads into the d tile
    gpsimd SWDGE: d += t  (accum-add DMA, SBUF->SBUF)
    DVE:    max tree (h, hm, m), E = is_ge(blocks 0..3 vs m),
            uv = blk01 - blk23 (fused u|v), w2 = v68 + u, p = wI * d
    ScalarE: v68 = 0.68*v (Copy w/ scale), wI = |w2 + bias_b| per block
    TensorE: ones-matmul reductions of p and d into two PSUM banks
  Readout: [1, 1024] f32 (psum_p | psum_d) -> DRAM; host: f64 sum, negate,
  / B.  Variable tile sizes (small first tile) shorten the DMA ramp.
"""

from contextlib import ExitStack

import numpy as np

P = 128
C = 5
DIST = (-0.5, -0.34, 0.0, 0.34, 0.5)
B = 4194304
NCORES = 8
ROWS_PER_CORE = B // NCORES  # 524288
GTOT = ROWS_PER_CORE // P    # 4096 rows per partition
GS = (1024, 1536, 1536)      # per-tile rows/partition/class-block
D_VIA_DMA = False             # False: d = on + t on DVE instead of accum DMA
assert sum(GS) == GTOT
NTILES = len(GS)

# class order in device layout; block b holds class PERM[b]
PERM = (4, 3, 0, 1, 2)
BIAS = tuple(-2.0 * DIST[c] for c in PERM)  # (-1.0, -0.68, 1.0, 0.68, -0.0)

_CACHE = {}


def _build_nc():
    import concourse.bacc as bacc
    import concourse.mybir as mybir
    import concourse.tile as tile

    F32 = mybir.dt.float32
    BF16 = mybir.dt.bfloat16

    nc = bacc.Bacc(target_bir_lowering=False)

    # Register activation-bias constants in the const-AP database (f32 keys,
    # as const_aps.scalar_like expects), mirroring Bass.__init__'s 0.0/1.0.
    for val in BIAS:
        if (F32, val) not in nc.const_aps.aps:
            tensor = nc.alloc_sbuf_tensor(f"const-f32-b{val}", [P, 1], F32)
            nc.gpsimd.memset(tensor.ap(), val)
            nc.const_aps.aps[(F32, val)] = tensor.ap()
    nc.all_engine_barrier()

    # host pre-arranges DRAM in per-tile layout [P, (c g)], concatenated
    # along the free dim in tile order
    t_in = nc.declare_dram_parameter("t", [P, C * GTOT], BF16, isOutput=False)
    on_in = nc.declare_dram_parameter("on", [P, C * GTOT], BF16, isOutput=False)
    out = nc.declare_dram_parameter("out", [1, 1024], F32, isOutput=True)

    offs = [C * sum(GS[:k]) for k in range(NTILES)]
    ones_bf16 = nc.const_aps.aps[(BF16, 1.0)]  # [128, 1] of 1.0

    with ExitStack() as ctx:
        tc = ctx.enter_context(tile.TileContext(nc))
        pool = ctx.enter_context(tc.tile_pool(name="work", bufs=2))
        psp = ctx.enter_context(tc.tile_pool(name="ps", bufs=1, space="PSUM"))
        outp = ctx.enter_context(tc.tile_pool(name="outp", bufs=1))
        ps_p = psp.tile([1, 512], F32)
        ps_d = psp.tile([1, 512], F32)

        GMAX = max(GS)
        st = {}
        mm = {"p_first": True, "d_first": True}
        n_mm = sum(C * g // 512 for g in GS)
        mm_done = {"p": 0, "d": 0}

        def dmm(k):
            """ones-matmul accumulate sum(d) into ps_d, 512-col slices."""
            d, g = st[k]["d"], GS[k]
            for j in range(C * g // 512):
                first = mm["d_first"]; mm["d_first"] = False
                mm_done["d"] += 1
                nc.tensor.matmul(
                    ps_d[:, :], ones_bf16, d[:, j * 512 : (j + 1) * 512],
                    start=first, stop=mm_done["d"] == n_mm,
                )

        def pmm(k, p):
            g = GS[k]
            for j in range(C * g // 512):
                first = mm["p_first"]; mm["p_first"] = False
                mm_done["p"] += 1
                nc.tensor.matmul(
                    ps_p[:, :], ones_bf16, p[:, j * 512 : (j + 1) * 512],
                    start=first, stop=mm_done["p"] == n_mm,
                )

        def phase_load(k):
            g = GS[k]
            t = pool.tile([P, C * GMAX], BF16, tag="t", name="t", bufs=2)
            nc.sync.dma_start(t[:, 0 : C * g], t_in[:, offs[k] : offs[k] + C * g])
            d = pool.tile([P, C * GMAX], BF16, tag="d", name="d", bufs=2)
            nc.scalar.dma_start(d[:, 0 : C * g], on_in[:, offs[k] : offs[k] + C * g])
            # d = -o + t  (gpsimd software DGE accum-add)
            if D_VIA_DMA:
                nc.gpsimd.dma_start(
                    d[:, 0 : C * g], t[:, 0 : C * g], accum_op=mybir.AluOpType.add
                )
            st[k] = {"t": t, "d": d}

        def phase_front(k):
            s = st[k]
            g = GS[k]
            t = s["t"]
            TT = nc.vector.tensor_tensor
            MAX = mybir.AluOpType.max

            # m = max over the 5 class blocks
            h = pool.tile([P, 2 * GMAX], BF16, tag="h", name="h", bufs=1)
            TT(h[:, 0 : 2 * g], t[:, 0 : 2 * g], t[:, 2 * g : 4 * g], op=MAX)
            hm = pool.tile([P, GMAX], BF16, tag="hm", name="hm", bufs=1)
            TT(hm[:, 0:g], h[:, 0:g], h[:, g : 2 * g], op=MAX)
            m = pool.tile([P, GMAX], BF16, tag="m", name="m", bufs=1)
            TT(m[:, 0:g], hm[:, 0:g], t[:, 4 * g : 5 * g], op=MAX)

            # E[b] = [t_b >= m] for blocks 0..3 (c4,c3,c0,c1)
            E = pool.tile([P, 4 * GMAX], BF16, tag="E", name="E", bufs=1)
            mv = m[:, 0:g].rearrange("p (x g) -> p x g", x=1)
            TT(
                E[:, 0 : 4 * g].rearrange("p (c g) -> p c g", g=g),
                t[:, 0 : 4 * g].rearrange("p (c g) -> p c g", g=g),
                mv.to_broadcast([P, 4, g]),
                op=mybir.AluOpType.is_ge,
            )

            # uv = (E_c4 - E_c0, E_c3 - E_c1) in one op on block pairs
            uv = pool.tile([P, 2 * GMAX], BF16, tag="uv", name="uv", bufs=1)
            TT(
                uv[:, 0 : 2 * g], E[:, 0 : 2 * g], E[:, 2 * g : 4 * g],
                op=mybir.AluOpType.subtract,
            )

            # v68 = 0.68 * v on ScalarE; w2 = v68 + u on DVE
            v68 = pool.tile([P, GMAX], BF16, tag="v68", name="v68", bufs=2)
            nc.scalar.mul(v68[:, 0:g], uv[:, g : 2 * g], 0.68)
            w2 = pool.tile([P, GMAX], BF16, tag="w2", name="w2", bufs=2)
            TT(w2[:, 0:g], v68[:, 0:g], uv[:, 0:g], op=mybir.AluOpType.add)

            # wI[b] = |w2 + bias_b|, contiguous per block (ScalarE)
            wI = pool.tile([P, C * GMAX], BF16, tag="wI", name="wI", bufs=2)
            for c in range(C):
                nc.scalar.activation(
                    wI[:, c * g : (c + 1) * g], w2[:, 0:g],
                    mybir.ActivationFunctionType.Abs,
                    bias=BIAS[c], scale=1.0,
                )
            s["wI"] = wI
            if not D_VIA_DMA:
                nc.vector.tensor_tensor(
                    s["d"][:, 0 : C * g], s["d"][:, 0 : C * g], t[:, 0 : C * g],
                    op=mybir.AluOpType.add,
                )
            dmm(k)

        def phase_back(k):
            s = st.pop(k)
            g = GS[k]
            wI, d = s["wI"], s["d"]
            p = pool.tile([P, C * GMAX], BF16, tag="p", name="p", bufs=2)
            nc.vector.tensor_tensor(
                p[:, 0 : C * g], wI[:, 0 : C * g], d[:, 0 : C * g],
                op=mybir.AluOpType.mult,
            )
            pmm(k, p)

        phase_load(0)
        phase_load(1)
        phase_front(0)
        phase_load(2)
        phase_front(1)
        phase_back(0)
        phase_front(2)
        phase_back(1)
        phase_back(2)

        res = outp.tile([1, 1024], F32)
        nc.scalar.copy(res[:, 0:512], ps_p[:, :])
        nc.scalar.copy(res[:, 512:1024], ps_d[:, :])
        nc.sync.dma_start(out[:, :], res[:, :])
    nc.finalize()
    return nc


def _get_nc():
    if "nc" not in _CACHE:
        _CACHE["nc"] = _build_nc()
    return _CACHE["nc"]


def _prep_inputs(output, target):
    """Per-core tile-layout bf16 arrays [P, (c g)] per tile; o negated."""
    from ml_dtypes import bfloat16

    def lay(x_core):
        parts = []
        r0 = 0
        for g in GS:
            x = x_core[r0 : r0 + P * g][:, list(PERM)].reshape(P, g, C)
            parts.append(x.transpose(0, 2, 1).reshape(P, C * g))
            r0 += P * g
        return np.ascontiguousarray(np.concatenate(parts, axis=1)).astype(bfloat16)

    o_sh = output.reshape(NCORES, ROWS_PER_CORE, C)
    t_sh = target.reshape(NCORES, ROWS_PER_CORE, C)
    return [{"t": lay(t_sh[i]), "on": lay(-o_sh[i])} for i in range(NCORES)]


def reduce_loss(res):
    total = 0.0
    for r in res.results:
        arr = r["out"].astype(np.float64).reshape(2, 512)
        total += 0.5 * float(arr[0].sum()) + float(arr[1].sum())
    return -total / B


def kernel(output, target, distance, _want_results=False):
    from concourse.bass_utils import run_bass_kernel_spmd

    output = np.asarray(output, dtype=np.float32)
    target = np.asarray(target, dtype=np.float32)
    distance = np.asarray(distance, dtype=np.float32)
    assert output.shape == (B, C) and target.shape == (B, C)
    assert np.allclose(distance, np.asarray(DIST, np.float32)), distance

    nc = _get_nc()
    in_maps = _prep_inputs(output, target)
    res = run_bass_kernel_spmd(nc, in_maps, core_ids=list(range(NCORES)))
    loss = np.float32(reduce_loss(res))
    if _want_results:
        return loss, res
    return loss


# revision 11
# speedup vs baseline: 1.4713x; 1.0216x over previous
"""Trainium2 Bass kernel for nn_CustomLoss (argmax-distance weighted loss).

reference:
    arg = argmax(target, axis=1)              # [B]
    delta = distance[arg]                     # [B]
    err = |distance[None,:] - delta[:,None]| + 1
    loss = sum((output - target) * err) / B

v7 design (data-parallel over 8 NeuronCores):
  Host: per-core slice, permute classes to PERM=(4,3,0,1,2), lay out DRAM
  in tile order [P, (c g)] per tile, cast f32->bf16 (halves HBM traffic,
  doubles DVE throughput; bf16 argmax ties cost ~5e-4 rel err), and
  pre-negate o so the device can form d via DMA accum-add.

  With one-hot E_c = [t_c >= m], m = max_c t_c, block order (c4,c3,c0,c1,c2):
    u = E_c4 - E_c0 = blk0 - blk2,  v = E_c3 - E_c1 = blk1 - blk3
    w2 = 2*delta = 0.68*v + u
    wI_b = |w2 - 2*dist_b|  (= 2*|delta - dist_b|)
    dneg = t - o  (= -(o - t))
    loss*B = -(0.5*sum(wI*dneg) + sum(dneg))

  Engines:
    sync HWDGE:   t loads (+ tiny bias-constant load)
    scalar HWDGE: (-o) loads into the d tile
    gpsimd SWDGE: d += t  accum-add DMA in <=2048-col chunks (bigger
                  accum transfers crash the runtime)
    DVE:    max tree (h, hm, m), E = is_ge(blocks 0..3 vs m),
            uv = blk01 - blk23 (fused u|v), w2 = v68 + u, p = wI * d
    ScalarE: v68 = 0.68*v (Copy w/ scale), wI = |w2 + bias_b| per block
             (bias read from the DMA'd constant tile, so no const-AP
             memset/barrier prologue)
    TensorE: ones-matmul reductions of p and d into two PSUM banks
  Last tile computes w2 via DVE stt (skips the v68 ScalarE hop) to
  shorten the exposed end-of-pipeline dependency chain.
  Readout: [1, 1024] f32 (psum_p | psum_d) -> DRAM; host: f64 sum, negate,
  / B.  Small first/last tiles shorten the DMA ramp and the tail.
"""

from contextlib import ExitStack

import numpy as np

P = 128
C = 5
DIST = (-0.5, -0.34, 0.0, 0.34, 0.5)
B = 4194304
NCORES = 8
ROWS_PER_CORE = B // NCORES  # 524288
GTOT = ROWS_PER_CORE // P    # 4096 rows per partition
GS = (512, 1536, 1536, 512)  # per-tile rows/partition/class-block
assert sum(GS) == GTOT
NTILES = len(GS)
ACHUNK = 2048                # accum-DMA chunk size (cols)

# class order in device layout; block b holds class PERM[b]
PERM = (4, 3, 0, 1, 2)
BIAS = tuple(-2.0 * DIST[c] for c in PERM)  # (-1.0, -0.68, 1.0, 0.68, -0.0)

_CACHE = {}


def _build_nc():
    import concourse.bacc as bacc
    import concourse.mybir as mybir
    import concourse.tile as tile

    F32 = mybir.dt.float32
    BF16 = mybir.dt.bfloat16

    nc = bacc.Bacc(target_bir_lowering=False)

    # host pre-arranges DRAM in per-tile layout [P, (c g)], concatenated
    # along the free dim in tile order
    t_in = nc.declare_dram_parameter("t", [P, C * GTOT], BF16, isOutput=False)
    on_in = nc.declare_dram_parameter("on", [P, C * GTOT], BF16, isOutput=False)
    bias_in = nc.declare_dram_parameter("bias", [P, C], F32, isOutput=False)
    out = nc.declare_dram_parameter("out", [1, 1024], F32, isOutput=True)

    offs = [C * sum(GS[:k]) for k in range(NTILES)]
    ones_bf16 = nc.const_aps.aps[(BF16, 1.0)]  # [128, 1] of 1.0

    with ExitStack() as ctx:
        tc = ctx.enter_context(tile.TileContext(nc))
        pool = ctx.enter_context(tc.tile_pool(name="work", bufs=2))
        psp = ctx.enter_context(tc.tile_pool(name="ps", bufs=1, space="PSUM"))
        outp = ctx.enter_context(tc.tile_pool(name="outp", bufs=1))
        ps_p = psp.tile([1, 512], F32)
        ps_d = psp.tile([1, 512], F32)
        bias = outp.tile([P, C], F32)
        nc.sync.dma_start(bias[:, :], bias_in[:, :])

        GMAX = max(GS)
        st = {}
        mm = {"p_first": True, "d_first": True}
        n_mm = sum(C * g // 512 for g in GS)
        mm_done = {"p": 0, "d": 0}

        def dmm(k):
            """ones-matmul accumulate sum(d) into ps_d, 512-col slices."""
            d, g = st[k]["d"], GS[k]
            for j in range(C * g // 512):
                first = mm["d_first"]; mm["d_first"] = False
                mm_done["d"] += 1
                nc.tensor.matmul(
                    ps_d[:, :], ones_bf16, d[:, j * 512 : (j + 1) * 512],
                    start=first, stop=mm_done["d"] == n_mm,
                )

        def pmm(k, p):
            g = GS[k]
            for j in range(C * g // 512):
                first = mm["p_first"]; mm["p_first"] = False
                mm_done["p"] += 1
                nc.tensor.matmul(
                    ps_p[:, :], ones_bf16, p[:, j * 512 : (j + 1) * 512],
                    start=first, stop=mm_done["p"] == n_mm,
                )

        def phase_load(k):
            g = GS[k]
            t = pool.tile([P, C * GMAX], BF16, tag="t", name="t", bufs=3)
            nc.sync.dma_start(t[:, 0 : C * g], t_in[:, offs[k] : offs[k] + C * g])
            d = pool.tile([P, C * GMAX], BF16, tag="d", name="d", bufs=3)
            nc.scalar.dma_start(d[:, 0 : C * g], on_in[:, offs[k] : offs[k] + C * g])
            # d = -o + t (gpsimd software DGE accum-add, chunked: large
            # accum transfers crash the runtime)
            for c0 in range(0, C * g, ACHUNK):
                c1 = min(c0 + ACHUNK, C * g)
                nc.gpsimd.dma_start(
                    d[:, c0:c1], t[:, c0:c1], accum_op=mybir.AluOpType.add
                )
            st[k] = {"t": t, "d": d}

        def phase_front(k):
            s = st[k]
            g = GS[k]
            t = s["t"]
            TT = nc.vector.tensor_tensor
            MAX = mybir.AluOpType.max

            # m = max over the 5 class blocks
            h = pool.tile([P, 2 * GMAX], BF16, tag="h", name="h", bufs=1)
            TT(h[:, 0 : 2 * g], t[:, 0 : 2 * g], t[:, 2 * g : 4 * g], op=MAX)
            hm = pool.tile([P, GMAX], BF16, tag="hm", name="hm", bufs=1)
            TT(hm[:, 0:g], h[:, 0:g], h[:, g : 2 * g], op=MAX)
            m = pool.tile([P, GMAX], BF16, tag="m", name="m", bufs=1)
            TT(m[:, 0:g], hm[:, 0:g], t[:, 4 * g : 5 * g], op=MAX)

            # E[b] = [t_b >= m] for blocks 0..3 (c4,c3,c0,c1)
            E = pool.tile([P, 4 * GMAX], BF16, tag="E", name="E", bufs=1)
            mv = m[:, 0:g].rearrange("p (x g) -> p x g", x=1)
            TT(
                E[:, 0 : 4 * g].rearrange("p (c g) -> p c g", g=g),
                t[:, 0 : 4 * g].rearrange("p (c g) -> p c g", g=g),
                mv.to_broadcast([P, 4, g]),
                op=mybir.AluOpType.is_ge,
            )

            # uv = (E_c4 - E_c0, E_c3 - E_c1) in one op on block pairs
            uv = pool.tile([P, 2 * GMAX], BF16, tag="uv", name="uv", bufs=1)
            TT(
                uv[:, 0 : 2 * g], E[:, 0 : 2 * g], E[:, 2 * g : 4 * g],
                op=mybir.AluOpType.subtract,
            )

            w2 = pool.tile([P, GMAX], BF16, tag="w2", name="w2", bufs=2)
            if k == NTILES - 1:
                # last tile: stay on DVE, skip the ScalarE v68 hop
                nc.vector.scalar_tensor_tensor(
                    w2[:, 0:g], uv[:, g : 2 * g], 0.68, uv[:, 0:g],
                    mybir.AluOpType.mult, mybir.AluOpType.add,
                )
            else:
                v68 = pool.tile([P, GMAX], BF16, tag="v68", name="v68", bufs=2)
                nc.scalar.mul(v68[:, 0:g], uv[:, g : 2 * g], 0.68)
                TT(w2[:, 0:g], v68[:, 0:g], uv[:, 0:g], op=mybir.AluOpType.add)

            # wI[b] = |w2 + bias_b|, contiguous per block (ScalarE)
            wI = pool.tile([P, C * GMAX], BF16, tag="wI", name="wI", bufs=2)
            for c in range(C):
                nc.scalar.activation(
                    wI[:, c * g : (c + 1) * g], w2[:, 0:g],
                    mybir.ActivationFunctionType.Abs,
                    bias=bias[:, c : c + 1], scale=1.0,
                )
            s["wI"] = wI
            dmm(k)

        def phase_back(k):
            s = st.pop(k)
            g = GS[k]
            wI, d = s["wI"], s["d"]
            p = pool.tile([P, C * GMAX], BF16, tag="p", name="p", bufs=2)
            nc.vector.tensor_tensor(
                p[:, 0 : C * g], wI[:, 0 : C * g], d[:, 0 : C * g],
                op=mybir.AluOpType.mult,
            )
            pmm(k, p)

        phase_load(0)
        phase_load(1)
        phase_front(0)
        phase_load(2)
        phase_front(1)
        phase_back(0)
        phase_load(3)
        phase_front(2)
        phase_back(1)
        phase_front(3)
        phase_back(2)
        phase_back(3)

        res = outp.tile([1, 1024], F32)
        nc.scalar.copy(res[:, 0:512], ps_p[:, :])
        nc.scalar.copy(res[:, 512:1024], ps_d[:, :])
        nc.sync.dma_start(out[:, :], res[:, :])
    nc.finalize()
    return nc


def _get_nc():
    if "nc" not in _CACHE:
        _CACHE["nc"] = _build_nc()
    return _CACHE["nc"]


def _prep_inputs(output, target):
    """Per-core tile-layout bf16 arrays [P, (c g)] per tile; o negated."""
    from ml_dtypes import bfloat16

    def lay(x_core):
        parts = []
        r0 = 0
        for g in GS:
            x = x_core[r0 : r0 + P * g][:, list(PERM)].reshape(P, g, C)
            parts.append(x.transpose(0, 2, 1).reshape(P, C * g))
            r0 += P * g
        return np.ascontiguousarray(np.concatenate(parts, axis=1)).astype(bfloat16)

    bias = np.tile(np.asarray(BIAS, np.float32), (P, 1))
    o_sh = output.reshape(NCORES, ROWS_PER_CORE, C)
    t_sh = target.reshape(NCORES, ROWS_PER_CORE, C)
    return [
        {"t": lay(t_sh[i]), "on": lay(-o_sh[i]), "bias": bias}
        for i in range(NCORES)
    ]


def reduce_loss(res):
    total = 0.0
    for r in res.results:
        arr = r["out"].astype(np.float64).reshape(2, 512)
        total += 0.5 * float(arr[0].sum()) + float(arr[1].sum())
    return -total / B


def kernel(output, target, distance, _want_results=False):
    from concourse.bass_utils import run_bass_kernel_spmd

    output = np.asarray(output, dtype=np.float32)
    target = np.asarray(target, dtype=np.float32)
    distance = np.asarray(distance, dtype=np.float32)
    assert output.shape == (B, C) and target.shape == (B, C)
    assert np.allclose(distance, np.asarray(DIST, np.float32)), distance

    nc = _get_nc()
    in_maps = _prep_inputs(output, target)
    res = run_bass_kernel_spmd(nc, in_maps, core_ids=list(range(NCORES)))
    loss = np.float32(reduce_loss(res))
    if _want_results:
        return loss, res
    return loss
